# revision 5
# baseline (speedup 1.0000x reference)
"""Trainium2 Bass kernel for nn_Attention_33354716021131 (v3).

Dense GQA attention (B=2, S=2048, D=4096, 32 q-heads / 8 kv-heads, head_dim
128, RoPE, causal softmax) tensor-parallel across 8 NeuronCores.

v3 over v2: single persistent pipeline with cross-phase interleaving
  [QKV b0] -> [QKV b1 (x) attn b0] -> [WO b0 (x) attn b1] -> [WO b1]
so the PE never sits at a phase boundary; one big-descriptor DMA per
256-token granule (host pre-tiles x); contiguous-block RoPE via per-head
[evens|odds] column permutation of wq/wk; AllGather fired per query-block
immediately; WO strips cached in SBUF and used for two tti passes.
"""
import math
import os

import numpy as np

N_CORES = 8
B = 2
S = 2048
DM = 4096
N_HEADS = 32
HD = 128
NQH = N_HEADS // N_CORES          # 4 q heads per core
HDQ = NQH * HD                    # 512
T = B * S                         # 4096 tokens
KC = DM // 128                    # 32 contraction chunks
NG = S // 256                     # 8 granules (256 tokens) per batch
NGT = S // 128                    # 16 token tiles per batch
QB = 512                          # query block for attention
NQB = S // QB                     # 4
SCALE = 1.0 / math.sqrt(HD)
ROPE_THETA = 10000.0

_CACHE = {}


def _consts():
    j = np.arange(HD // 2)
    inv = 1.0 / (ROPE_THETA ** (2 * j / HD))          # [64]
    pos = np.arange(S).reshape(NGT, 128)              # [16, 128]
    ang = pos[:, :, None] * inv[None, None, :]        # [16, 128, 64]
    cos = np.cos(ang).astype(np.float32)
    sin = np.sin(ang).astype(np.float32)
    # layout [128 part, tt, h, 64] -> [128, 16*256] (same table per head)
    cos4 = np.tile(cos.transpose(1, 0, 2)[:, :, None, :], (1, 1, NQH, 1))
    sin4 = np.tile(sin.transpose(1, 0, 2)[:, :, None, :], (1, 1, NQH, 1))
    cos4 = np.ascontiguousarray(cos4.reshape(128, NGT * NQH * 64))
    sin4 = np.ascontiguousarray(sin4.reshape(128, NGT * NQH * 64))
    tri = (np.arange(128)[:, None] <= np.arange(128)[None, :]).astype(np.float32)
    ident = np.eye(128, dtype=np.float32)
    ones = np.ones((128, 128), np.float32)
    return cos4, sin4, tri, ident, ones


def _build(sim=False):
    import concourse.mybir as mybir
    import concourse.tile as tile
    from concourse import bacc

    F32 = mybir.dt.float32
    BF16 = mybir.dt.bfloat16

    nc = bacc.Bacc("TRN2", target_bir_lowering=False, debug=False,
                   num_devices=N_CORES)

    xg = nc.dram_tensor("xg", [B * NG * 128, KC * 256], BF16,
                        kind="ExternalInput")
    wqh = nc.dram_tensor("wqh", [128, KC * HDQ], BF16, kind="ExternalInput")
    wkvh = nc.dram_tensor("wkvh", [128, KC * 256], BF16, kind="ExternalInput")
    woh = nc.dram_tensor("woh", [128, KC * HDQ], BF16, kind="ExternalInput")
    cosc = nc.dram_tensor("cosc", [128, NGT * 256], BF16, kind="ExternalInput")
    sinc = nc.dram_tensor("sinc", [128, NGT * 256], BF16, kind="ExternalInput")
    tric = nc.dram_tensor("tric", [128, 128], BF16, kind="ExternalInput")
    identc = nc.dram_tensor("identc", [128, 128], BF16, kind="ExternalInput")
    onesc = nc.dram_tensor("onesc", [128, 128], BF16, kind="ExternalInput")

    y = nc.dram_tensor("y", [T, HDQ], BF16, kind="ExternalOutput")

    rg = [list(range(N_CORES))]

    with tile.TileContext(nc) as tc:
        with (
            tc.tile_pool(name="dram", bufs=1, space="DRAM") as dram,
            tc.tile_pool(name="const", bufs=1) as cp,
            tc.tile_pool(name="wqkv", bufs=1) as wpool,
            tc.tile_pool(name="batch", bufs=1) as bp,
            tc.tile_pool(name="rwp", bufs=2) as rwp,
            tc.tile_pool(name="qrp", bufs=2) as qrp,
            tc.tile_pool(name="tmp", bufs=2) as tmp,
            tc.tile_pool(name="wa", bufs=1) as wa,
            tc.tile_pool(name="ptp", bufs=3) as ptp,
            tc.tile_pool(name="accp", bufs=2) as accp,
            tc.tile_pool(name="otsb", bufs=3) as otsb,
            tc.tile_pool(name="ywp", bufs=2) as ywp,
            tc.tile_pool(name="ps_s", bufs=3, space="PSUM") as ps_s,
            tc.tile_pool(name="ps_o", bufs=2, space="PSUM") as ps_o,
        ):
            # ---- weights / consts (first-needed chunks first) ----
            wq_sb = wpool.tile([128, KC * HDQ], BF16, tag="wq")
            wkv_sb = wpool.tile([128, KC * 256], BF16, tag="wkv")
            wo_sb = wpool.tile([128, KC * HDQ], BF16, tag="wo")

            def load_weights():
                nc.scalar.dma_start(out=wkv_sb[:, 0:8 * 256],
                                    in_=wkvh.ap()[:, 0:8 * 256])
                nc.sync.dma_start(out=wq_sb[:, 2 * HDQ:8 * HDQ],
                                  in_=wqh.ap()[:, 2 * HDQ:8 * HDQ])
                for ch in range(1, 4):
                    csl = slice(ch * 8 * HDQ, (ch + 1) * 8 * HDQ)
                    nc.sync.dma_start(out=wq_sb[:, csl], in_=wqh.ap()[:, csl])
                    ksl = slice(ch * 8 * 256, (ch + 1) * 8 * 256)
                    nc.scalar.dma_start(out=wkv_sb[:, ksl],
                                        in_=wkvh.ap()[:, ksl])
                nc.scalar.dma_start(out=cos_sb[:], in_=cosc.ap())
                nc.scalar.dma_start(out=sin_sb[:], in_=sinc.ap())
                nc.scalar.dma_start(out=tri_sb[:], in_=tric.ap())
                nc.scalar.dma_start(out=id_sb[:], in_=identc.ap())
                nc.scalar.dma_start(out=ones_sb[:], in_=onesc.ap())
                for ch in range(4):
                    csl = slice(ch * 8 * HDQ, (ch + 1) * 8 * HDQ)
                    eng = nc.scalar if ch % 2 == 0 else nc.sync
                    eng.dma_start(out=wo_sb[:, csl], in_=woh.ap()[:, csl])
            cos_sb = cp.tile([128, NGT * 256], BF16, tag="cos")
            sin_sb = cp.tile([128, NGT * 256], BF16, tag="sin")
            tri_sb = cp.tile([128, 128], BF16, tag="tri")
            id_sb = cp.tile([128, 128], BF16, tag="id")
            ones_sb = cp.tile([128, 128], BF16, tag="ones")

            qTall = [bp.tile([128, NQH * S], BF16, tag=f"qTall{i}",
                             name=f"qTall{i}") for i in range(2)]
            kT = [bp.tile([128, S], BF16, tag=f"kT{i}", name=f"kT{i}")
                  for i in range(2)]
            v_nat = [bp.tile([128, S], BF16, tag=f"v_nat{i}",
                             name=f"v_nat{i}") for i in range(2)]

            oT_h = [[dram.tile([HDQ, QB], BF16, name=f"oT_h{b}_{qb}")
                     for qb in range(NQB)] for b in range(B)]
            oT_F = [[dram.tile([DM, QB], BF16,
                               addr_space="Local" if sim else "Shared",
                               name=f"oT_F{b}_{qb}") for qb in range(NQB)]
                    for b in range(B)]

            env = dict(
                nc=nc, tc=tc, mybir=mybir, F32=F32, BF16=BF16,
                xg=xg, wq_sb=wq_sb, wkv_sb=wkv_sb,
                cos_sb=cos_sb, sin_sb=sin_sb, tri_sb=tri_sb, id_sb=id_sb,
                ones_sb=ones_sb, qTall=qTall, kT=kT, v_nat=v_nat,
                oT_h=oT_h, oT_F=oT_F, rg=rg, sim=sim,
                rwp=rwp, qrp=qrp, tmp=tmp, wa=wa, ptp=ptp, accp=accp,
                otsb=otsb, ywp=ywp, ps_s=ps_s, ps_o=ps_o, y=y,
            )

            with (
                tc.tile_pool(name="xgp", bufs=2) as xgp,
                tc.tile_pool(name="ps_q", bufs=1, space="PSUM") as ps_q,
                tc.tile_pool(name="ps_kv", bufs=1, space="PSUM") as ps_kv,
                tc.tile_pool(name="ps_T", bufs=1, space="PSUM") as ps_T,
            ):
                env.update(xgp=xgp, ps_q=ps_q, ps_kv=ps_kv, ps_T=ps_T)
                xg_tiles = {}
                env["xg_tiles"] = xg_tiles

                def load_xg(b, g):
                    t = xgp.tile([128, KC * 256], BF16, tag="xg")
                    r0 = (b * NG + g) * 128
                    half = KC * 128
                    eng = nc.sync if g % 2 == 0 else nc.scalar
                    eng.dma_start(out=t[:, 0:half],
                                  in_=xg.ap()[r0:r0 + 128, 0:half])
                    eng.dma_start(out=t[:, half:],
                                  in_=xg.ap()[r0:r0 + 128, half:])
                    xg_tiles[(b, g)] = t

                t0 = xgp.tile([128, KC * 256], BF16, tag="xg")
                q4 = KC * 64
                nc.sync.dma_start(out=t0[:, 0:q4],
                                  in_=xg.ap()[0:128, 0:q4])
                nc.scalar.dma_start(out=t0[:, 2 * q4:3 * q4],
                                    in_=xg.ap()[0:128, 2 * q4:3 * q4])
                nc.sync.dma_start(out=wq_sb[:, 0:2 * HDQ],
                                  in_=wqh.ap()[:, 0:2 * HDQ])
                nc.sync.dma_start(out=t0[:, q4:2 * q4],
                                  in_=xg.ap()[0:128, q4:2 * q4])
                nc.scalar.dma_start(out=t0[:, 3 * q4:],
                                    in_=xg.ap()[0:128, 3 * q4:])
                xg_tiles[(0, 0)] = t0
                load_xg(0, 1)
                load_weights()

                # ---- phase A: QKV b0 alone ----
                for g in range(NG):
                    _qkv_granule(env, 0, g)

                # ---- phase B: QKV b1 interleaved with attn b0 ----
                for i in range(16):
                    g, tt = i // 2, i % 2
                    _qkv_granule_half(env, 1, g, tt)
                    _attn_unit(env, 0, i // 4, i % 4)
                    if i % 4 == 3:
                        _emit_ag(env, 0, i // 4)

            # QKV psum + xg pools closed; open WO pools in freed space
            with (
                tc.tile_pool(name="stp", bufs=34) as stp,
                tc.tile_pool(name="ps_y", bufs=2, space="PSUM") as ps_y,
            ):
                env.update(wo_sb=wo_sb, stp=stp, ps_y=ps_y)
                _load_strips(env, 0, 0)

                # ---- phase C: attn b1 interleaved with WO b0 + early WO b1
                qb_order = [2, 3, 1, 0]
                wo_stream = [(0, qb, half) for qb in range(NQB)
                             for half in range(2)]
                wo_stream += [(1, qb, half) for qb in (2, 3)
                              for half in range(2)]
                wi = 0
                for i in range(16):
                    _attn_unit(env, 1, qb_order[i // 4], i % 4)
                    if i % 4 == 3:
                        _emit_ag(env, 1, qb_order[i // 4])
                    # ~12 wo half-units across 16 attn units
                    while wi < len(wo_stream) and wi * 16 <= i * 12:
                        _wo_half(env, *wo_stream[wi])
                        wi += 1
                while wi < len(wo_stream):
                    _wo_half(env, *wo_stream[wi])
                    wi += 1

                # ---- phase D: rest of WO b1 ----
                for qb in (1, 0):
                    for half in range(2):
                        _wo_half(env, 1, qb, half)

    nc.compile()
    return nc


def _qkv_granule(env, b, g):
    for tt in range(2):
        _qkv_granule_half(env, b, g, tt)


def _qkv_granule_half(env, b, g, tt):
    """One 128-token tile: q pass, kv pass, rope, transposes."""
    nc = env["nc"]
    F32, BF16 = env["F32"], env["BF16"]
    par = b % 2
    gt = g * 2 + tt                    # token tile index within batch
    pos = gt * 128

    if tt == 0:
        # prefetch granule g+3 of this batch, or early granules of b+1
        pg = g + 2
        pb = b
        if pg >= NG:
            pb, pg = b + 1, pg - NG
        if pb < B:
            xgp, xg, xg_tiles = env["xgp"], env["xg"], env["xg_tiles"]
            t = xgp.tile([128, KC * 256], BF16, tag="xg")
            r0 = (pb * NG + pg) * 128
            half = KC * 128
            eng = nc.sync if pg % 2 == 0 else nc.scalar
            eng.dma_start(out=t[:, 0:half], in_=xg.ap()[r0:r0 + 128, 0:half])
            eng.dma_start(out=t[:, half:], in_=xg.ap()[r0:r0 + 128, half:])
            xg_tiles[(pb, pg)] = t

    xg_t = env["xg_tiles"][(b, g)]
    wq_sb, wkv_sb = env["wq_sb"], env["wkv_sb"]

    # q pass: out [128 tok, 512 dq], accumulate over 32 chunks
    psq = env["ps_q"].tile([128, HDQ], F32, tag="psq")
    for kc in range(KC):
        nc.tensor.matmul(
            psq[:], xg_t[:, kc * 256 + tt * 128: kc * 256 + (tt + 1) * 128],
            wq_sb[:, kc * HDQ:(kc + 1) * HDQ],
            start=(kc == 0), stop=(kc == KC - 1),
        )
    # kv pass: out [128 tok, 256 (k|v)]
    pskv = env["ps_kv"].tile([128, 256], F32, tag="kv",
                             padded_shape=[128, 512])
    for kc in range(KC):
        nc.tensor.matmul(
            pskv[:], xg_t[:, kc * 256 + tt * 128: kc * 256 + (tt + 1) * 128],
            wkv_sb[:, kc * 256:(kc + 1) * 256],
            start=(kc == 0), stop=(kc == KC - 1),
        )

    # drains: copies + rope (scalar/vector), then PE transposes
    rwp, qrp, tmp = env["rwp"], env["qrp"], env["tmp"]
    cos_sb, sin_sb, id_sb = env["cos_sb"], env["sin_sb"], env["id_sb"]
    rq = rwp.tile([128, HDQ], BF16, tag="rq")
    nc.scalar.copy(rq[:], psq[:])
    rkv = rwp.tile([128, 256], BF16, tag="rkv")
    nc.scalar.copy(rkv[:], pskv[:])
    nc.scalar.copy(env["v_nat"][par][:, pos:pos + 128], rkv[:, 128:256])

    # contiguous-block rope: head slice u<64 -> x0 (even dims), u>=64 -> x1
    q_rot = qrp.tile([128, HDQ], BF16, tag="qr")
    csl = slice(gt * 256, (gt + 1) * 256)
    cosv = cos_sb[:, csl].rearrange("p (h d) -> p h d", h=NQH)
    sinv = sin_sb[:, csl].rearrange("p (h d) -> p h d", h=NQH)
    rqv = rq[:].rearrange("p (h half d) -> p h half d", h=NQH, half=2)
    rov = q_rot[:].rearrange("p (h half d) -> p h half d", h=NQH, half=2)
    x0 = rqv[:, :, 0, :]
    x1 = rqv[:, :, 1, :]
    m0 = tmp.tile([128, 256], BF16, tag="m0", name="m0")
    m1 = tmp.tile([128, 256], BF16, tag="m1", name="m1")
    m0v = m0[:].rearrange("p (h d) -> p h d", h=NQH)
    m1v = m1[:].rearrange("p (h d) -> p h d", h=NQH)
    nc.vector.tensor_mul(m0v, x0, cosv)
    nc.vector.tensor_mul(m1v, x1, sinv)
    nc.vector.tensor_sub(rov[:, :, 0, :], m0v, m1v)
    m2 = tmp.tile([128, 256], BF16, tag="m0", name="m2")
    m3 = tmp.tile([128, 256], BF16, tag="m1", name="m3")
    m2v = m2[:].rearrange("p (h d) -> p h d", h=NQH)
    m3v = m3[:].rearrange("p (h d) -> p h d", h=NQH)
    nc.vector.tensor_mul(m2v, x0, sinv)
    nc.vector.tensor_mul(m3v, x1, cosv)
    nc.vector.tensor_add(rov[:, :, 1, :], m2v, m3v)

    k_rot = qrp.tile([128, 128], BF16, tag="kr")
    kc0 = cos_sb[:, gt * 256:gt * 256 + 64]
    ks0 = sin_sb[:, gt * 256:gt * 256 + 64]
    kx0 = rkv[:, 0:64]
    kx1 = rkv[:, 64:128]
    km0 = tmp.tile([128, 64], BF16, tag="km0", name="km0")
    km1 = tmp.tile([128, 64], BF16, tag="km1", name="km1")
    nc.vector.tensor_mul(km0[:], kx0, kc0)
    nc.vector.tensor_mul(km1[:], kx1, ks0)
    nc.vector.tensor_sub(k_rot[:, 0:64], km0[:], km1[:])
    km2 = tmp.tile([128, 64], BF16, tag="km0", name="km2")
    km3 = tmp.tile([128, 64], BF16, tag="km1", name="km3")
    nc.vector.tensor_mul(km2[:], kx0, ks0)
    nc.vector.tensor_mul(km3[:], kx1, kc0)
    nc.vector.tensor_add(k_rot[:, 64:128], km2[:], km3[:])

    # transposes (q heads + k) into one packed psum bank
    tT = env["ps_T"].tile([128, 640], BF16, tag="T", padded_shape=[128, 1024])
    for h in range(NQH):
        nc.tensor.transpose(tT[:, h * 128:(h + 1) * 128],
                            q_rot[:, h * 128:(h + 1) * 128], id_sb[:])
    nc.tensor.transpose(tT[:, 512:640], k_rot[:], id_sb[:])
    nc.scalar.copy(
        env["qTall"][par][:].rearrange("p (h s) -> p h s", h=NQH)
        [:, :, pos:pos + 128],
        tT[:, 0:512].rearrange("p (h t) -> p h t", h=NQH),
    )
    nc.scalar.copy(env["kT"][par][:, pos:pos + 128], tT[:, 512:640])


def _attn_unit(env, b, qb, h):
    nc, mybir = env["nc"], env["mybir"]
    F32, BF16 = env["F32"], env["BF16"]
    par = b % 2
    qTall, kTt, v_natt = env["qTall"][par], env["kT"][par], env["v_nat"][par]
    tri_sb, ones_sb = env["tri_sb"], env["ones_sb"]
    ps_s, ps_o = env["ps_s"], env["ps_o"]
    ptp, accp, wa, otsb = env["ptp"], env["accp"], env["wa"], env["otsb"]

    q0 = qb * QB
    kt_max = (q0 + QB) // 128 - 1
    oT = ps_o.tile([128, QB], F32, tag="oT")
    acc = accp.tile([128, QB], BF16, tag="acc", name="acc")

    sTs = {}

    def emit_s(kt):
        off = max(0, kt * 128 - q0)
        qs = slice(h * S + q0 + off, h * S + q0 + QB)
        sT = ps_s.tile([128, QB], F32, tag="sT", name="sT")
        nc.tensor.matmul(
            sT[:, off:QB],
            kTt[:, kt * 128:(kt + 1) * 128],
            qTall[:, qs],
            start=True, stop=True,
        )
        sTs[kt] = (sT, off)

    emit_s(0)
    if kt_max >= 1:
        emit_s(1)
    for kt in range(kt_max + 1):
        if kt + 2 <= kt_max:
            emit_s(kt + 2)
        sT, off = sTs.pop(kt)
        psl = slice(off, QB)
        pT = ptp.tile([128, QB], BF16, tag="pT", name="pT")
        nc.scalar.activation(
            pT[:, psl], sT[:, psl],
            mybir.ActivationFunctionType.Exp,
            scale=SCALE,
        )
        if kt * 128 >= q0:
            nc.vector.tensor_mul(
                pT[:, off:off + 128],
                pT[:, off:off + 128],
                tri_sb[:],
            )
        nc.tensor.matmul(
            oT[:, psl],
            v_natt[:, kt * 128:(kt + 1) * 128],
            pT[:, psl],
            start=(kt == 0), stop=(kt == kt_max),
        )
        if kt == 0:
            nc.vector.tensor_copy(acc[:], pT[:])
        else:
            nc.vector.tensor_add(acc[:, psl], acc[:, psl], pT[:, psl])

    sums = ps_s.tile([128, QB], F32, tag="sT", name="sums")
    nc.tensor.matmul(sums[:], ones_sb[:], acc[:], start=True, stop=True)
    rec = wa.tile([128, QB], F32, tag="rec")
    scr = wa.tile([128, QB], F32, tag="scr")
    nc.vector.reciprocal_approx_accurate(rec[:], sums[:], scr[:])
    oT_sb = otsb.tile([128, QB], BF16, tag="oT_sb")
    nc.vector.tensor_mul(oT_sb[:], oT[:], rec[:])
    nc.sync.dma_start(
        out=env["oT_h"][b][qb][:][h * 128:(h + 1) * 128, :],
        in_=oT_sb[:],
    )


def _emit_ag(env, b, qb):
    nc, mybir = env["nc"], env["mybir"]
    if not env["sim"]:
        nc.gpsimd.collective_compute(
            "AllGather", mybir.AluOpType.bypass,
            replica_groups=env["rg"],
            ins=[env["oT_h"][b][qb][:].opt()],
            outs=[env["oT_F"][b][qb][:].opt()],
        )
    else:
        for c in range(N_CORES):
            nc.sync.dma_start(
                out=env["oT_F"][b][qb][:][c * HDQ:(c + 1) * HDQ, :],
                in_=env["oT_h"][b][qb][:],
            )


def _load_strips(env, b, qb, three_q=False):
    nc, BF16 = env["nc"], env["BF16"]
    stp, oT_F = env["stp"], env["oT_F"]
    key = ("strips", b, qb)
    if key not in env:
        if three_q:
            engs = [nc.sync, nc.scalar, nc.gpsimd]
        else:
            # keep the scalar queue free for exp during phase C
            engs = [nc.sync, nc.gpsimd]
        strips = []
        for hc in range(KC):
            strip = stp.tile([128, QB], BF16, tag="strip")
            engs[hc % len(engs)].dma_start(
                out=strip[:],
                in_=oT_F[b][qb][:][hc * 128:(hc + 1) * 128, :],
            )
            strips.append(strip)
        env[key] = strips
    return env[key]


def _wo_half(env, b, qb, half):
    """WO for 512 tokens of (b, qb): half 0 computes tti 0,1 and loads the
    strips; half 1 reuses the cached strips for tti 2,3."""
    nc = env["nc"]
    F32, BF16 = env["F32"], env["BF16"]
    stp, ps_y, ywp, wo_sb = env["stp"], env["ps_y"], env["ywp"], env["wo_sb"]
    y, oT_F = env["y"], env["oT_F"]

    strips = _load_strips(env, b, qb)

    psy = [ps_y.tile([128, HDQ], F32, tag="psy", name=f"psy{i}")
           for i in range(2)]
    for hc in range(KC):
        strip = strips[hc]
        for i in range(2):
            tti = half * 2 + i
            nc.tensor.matmul(
                psy[i][:],
                strip[:, tti * 128:(tti + 1) * 128],
                wo_sb[:, hc * HDQ:(hc + 1) * HDQ],
                start=(hc == 0), stop=(hc == KC - 1),
            )
    for i in range(2):
        tti = half * 2 + i
        y_sb = ywp.tile([128, HDQ], BF16, tag="y_sb")
        if i == 0:
            nc.scalar.copy(y_sb[:], psy[i][:])
        else:
            nc.vector.tensor_copy(y_sb[:], psy[i][:])
        row = b * S + qb * QB + tti * 128
        nc.sync.dma_start(out=y.ap()[row:row + 128, :], in_=y_sb[:])


def _in_maps(x, wq, wk, wv, wo):
    import concourse.mybir as mybir
    np_bf16 = mybir.dt.np(mybir.dt.bfloat16)

    x2 = np.asarray(x, dtype=np.float32).reshape(B, NG, 256, KC, 128)
    # xg[(b g) p, kc t] = x[b, g*256+t, kc*128+p]
    xgf = np.ascontiguousarray(x2.transpose(0, 1, 4, 3, 2)).reshape(
        B * NG * 128, KC * 256).astype(np_bf16)

    perm = np.concatenate([np.arange(0, HD, 2), np.arange(1, HD, 2)])
    wq = np.asarray(wq, np.float32)
    wk = np.asarray(wk, np.float32)
    wv = np.asarray(wv, np.float32)
    wo = np.asarray(wo, np.float32)
    cos4, sin4, tri, ident, ones = _consts()

    maps = []
    for c in range(N_CORES):
        wq_c = wq[:, c * HDQ:(c + 1) * HDQ].reshape(DM, NQH, HD)
        wq_c = wq_c[:, :, perm].reshape(DM, HDQ)
        # wqh[p, kc*HDQ + u] = wq_c[kc*128+p, u]
        wqh = np.ascontiguousarray(
            wq_c.reshape(KC, 128, HDQ).transpose(1, 0, 2)).reshape(
            128, KC * HDQ).astype(np_bf16)

        wk_c = wk[:, c * HD:(c + 1) * HD][:, perm]
        wv_c = wv[:, c * HD:(c + 1) * HD]
        wkv_c = np.concatenate([wk_c, wv_c], axis=1)      # [DM, 256]
        wkvh = np.ascontiguousarray(
            wkv_c.reshape(KC, 128, 256).transpose(1, 0, 2)).reshape(
            128, KC * 256).astype(np_bf16)

        wo_c = wo[:, c * HDQ:(c + 1) * HDQ]
        woh = np.ascontiguousarray(
            wo_c.reshape(KC, 128, HDQ).transpose(1, 0, 2)).reshape(
            128, KC * HDQ).astype(np_bf16)

        maps.append({
            "xg": xgf,
            "wqh": wqh, "wkvh": wkvh, "woh": woh,
            "cosc": cos4.astype(np_bf16), "sinc": sin4.astype(np_bf16),
            "tric": tri.astype(np_bf16),
            "identc": ident.astype(np_bf16), "onesc": ones.astype(np_bf16),
        })
    return maps


def kernel(x, wq, wk, wv, wo, start_pos=0, **_unused):
    from concourse import bass_utils

    assert int(np.asarray(start_pos)) == 0
    in_maps = _in_maps(x, wq, wk, wv, wo)

    if "nc" not in _CACHE:
        _CACHE["nc"] = _build()
    nc = _CACHE["nc"]

    res = bass_utils.run_bass_kernel_spmd(
        nc, in_maps, core_ids=list(range(N_CORES)),
        trace=bool(int(os.environ.get("KERNEL_TRACE", "0") or 0)),
    )
    _CACHE["last_result"] = res

    out = np.empty((T, DM), np.float32)
    for c in range(N_CORES):
        out[:, c * HDQ:(c + 1) * HDQ] = np.asarray(
            res.results[c]["y"], dtype=np.float32)
    return out.reshape(B, S, DM)


# revision 6
# speedup vs baseline: 1.0064x; 1.0064x over previous
"""Trainium2 Bass kernel for nn_Attention_33354716021131 (v3).

Dense GQA attention (B=2, S=2048, D=4096, 32 q-heads / 8 kv-heads, head_dim
128, RoPE, causal softmax) tensor-parallel across 8 NeuronCores.

v3 over v2: single persistent pipeline with cross-phase interleaving
  [QKV b0] -> [QKV b1 (x) attn b0] -> [WO b0 (x) attn b1] -> [WO b1]
so the PE never sits at a phase boundary; one big-descriptor DMA per
256-token granule (host pre-tiles x); contiguous-block RoPE via per-head
[evens|odds] column permutation of wq/wk; AllGather fired per query-block
immediately; WO strips cached in SBUF and used for two tti passes.
"""
import math
import os

import numpy as np

N_CORES = 8
B = 2
S = 2048
DM = 4096
N_HEADS = 32
HD = 128
NQH = N_HEADS // N_CORES          # 4 q heads per core
HDQ = NQH * HD                    # 512
T = B * S                         # 4096 tokens
KC = DM // 128                    # 32 contraction chunks
NG = S // 256                     # 8 granules (256 tokens) per batch
NGT = S // 128                    # 16 token tiles per batch
QB = 512                          # query block for attention
NQB = S // QB                     # 4
SCALE = 1.0 / math.sqrt(HD)
ROPE_THETA = 10000.0

_CACHE = {}


def _consts():
    j = np.arange(HD // 2)
    inv = 1.0 / (ROPE_THETA ** (2 * j / HD))          # [64]
    pos = np.arange(S).reshape(NGT, 128)              # [16, 128]
    ang = pos[:, :, None] * inv[None, None, :]        # [16, 128, 64]
    cos = np.cos(ang).astype(np.float32)
    sin = np.sin(ang).astype(np.float32)
    # layout [128 part, tt, h, 64] -> [128, 16*256] (same table per head)
    cos4 = np.tile(cos.transpose(1, 0, 2)[:, :, None, :], (1, 1, NQH, 1))
    sin4 = np.tile(sin.transpose(1, 0, 2)[:, :, None, :], (1, 1, NQH, 1))
    cos4 = np.ascontiguousarray(cos4.reshape(128, NGT * NQH * 64))
    sin4 = np.ascontiguousarray(sin4.reshape(128, NGT * NQH * 64))
    tri = (np.arange(128)[:, None] <= np.arange(128)[None, :]).astype(np.float32)
    ident = np.eye(128, dtype=np.float32)
    ones = np.ones((128, 128), np.float32)
    return cos4, sin4, tri, ident, ones


def _build(sim=False):
    import concourse.mybir as mybir
    import concourse.tile as tile
    from concourse import bacc

    F32 = mybir.dt.float32
    BF16 = mybir.dt.bfloat16

    nc = bacc.Bacc("TRN2", target_bir_lowering=False, debug=False,
                   num_devices=N_CORES)

    xg = nc.dram_tensor("xg", [B * NG * 128, KC * 256], BF16,
                        kind="ExternalInput")
    wqh = nc.dram_tensor("wqh", [128, KC * HDQ], BF16, kind="ExternalInput")
    wkvh = nc.dram_tensor("wkvh", [128, KC * 256], BF16, kind="ExternalInput")
    woh = nc.dram_tensor("woh", [128, KC * HDQ], BF16, kind="ExternalInput")
    cosc = nc.dram_tensor("cosc", [128, NGT * 256], BF16, kind="ExternalInput")
    sinc = nc.dram_tensor("sinc", [128, NGT * 256], BF16, kind="ExternalInput")
    tric = nc.dram_tensor("tric", [128, 128], BF16, kind="ExternalInput")
    identc = nc.dram_tensor("identc", [128, 128], BF16, kind="ExternalInput")
    onesc = nc.dram_tensor("onesc", [128, 128], BF16, kind="ExternalInput")

    y = nc.dram_tensor("y", [T, HDQ], BF16, kind="ExternalOutput")

    rg = [list(range(N_CORES))]

    with tile.TileContext(nc) as tc:
        with (
            tc.tile_pool(name="dram", bufs=1, space="DRAM") as dram,
            tc.tile_pool(name="const", bufs=1) as cp,
            tc.tile_pool(name="wqkv", bufs=1) as wpool,
            tc.tile_pool(name="batch", bufs=1) as bp,
            tc.tile_pool(name="rwp", bufs=2) as rwp,
            tc.tile_pool(name="qrp", bufs=2) as qrp,
            tc.tile_pool(name="tmp", bufs=2) as tmp,
            tc.tile_pool(name="wa", bufs=1) as wa,
            tc.tile_pool(name="ptp", bufs=3) as ptp,
            tc.tile_pool(name="accp", bufs=2) as accp,
            tc.tile_pool(name="otsb", bufs=3) as otsb,
            tc.tile_pool(name="ywp", bufs=2) as ywp,
            tc.tile_pool(name="ps_s", bufs=3, space="PSUM") as ps_s,
            tc.tile_pool(name="ps_o", bufs=2, space="PSUM") as ps_o,
        ):
            # ---- weights / consts (first-needed chunks first) ----
            wq_sb = wpool.tile([128, KC * HDQ], BF16, tag="wq")
            wkv_sb = wpool.tile([128, KC * 256], BF16, tag="wkv")
            wo_sb = wpool.tile([128, KC * HDQ], BF16, tag="wo")

            def load_weights():
                nc.gpsimd.dma_start(out=wkv_sb[:, 0:8 * 256],
                                    in_=wkvh.ap()[:, 0:8 * 256])
                nc.gpsimd.dma_start(out=cos_sb[:], in_=cosc.ap())
                nc.gpsimd.dma_start(out=sin_sb[:], in_=sinc.ap())
                nc.gpsimd.dma_start(out=id_sb[:], in_=identc.ap())
                nc.gpsimd.dma_start(out=tri_sb[:], in_=tric.ap())
                nc.gpsimd.dma_start(out=ones_sb[:], in_=onesc.ap())
                nc.sync.dma_start(out=wq_sb[:, 2 * HDQ:8 * HDQ],
                                  in_=wqh.ap()[:, 2 * HDQ:8 * HDQ])
                for ch in range(1, 4):
                    csl = slice(ch * 8 * HDQ, (ch + 1) * 8 * HDQ)
                    nc.sync.dma_start(out=wq_sb[:, csl], in_=wqh.ap()[:, csl])
                    ksl = slice(ch * 8 * 256, (ch + 1) * 8 * 256)
                    nc.gpsimd.dma_start(out=wkv_sb[:, ksl],
                                        in_=wkvh.ap()[:, ksl])
                for ch in range(4):
                    csl = slice(ch * 8 * HDQ, (ch + 1) * 8 * HDQ)
                    eng = nc.gpsimd if ch % 2 == 0 else nc.sync
                    eng.dma_start(out=wo_sb[:, csl], in_=woh.ap()[:, csl])
            cos_sb = cp.tile([128, NGT * 256], BF16, tag="cos")
            sin_sb = cp.tile([128, NGT * 256], BF16, tag="sin")
            tri_sb = cp.tile([128, 128], BF16, tag="tri")
            id_sb = cp.tile([128, 128], BF16, tag="id")
            ones_sb = cp.tile([128, 128], BF16, tag="ones")

            qTall = [bp.tile([128, NQH * S], BF16, tag=f"qTall{i}",
                             name=f"qTall{i}") for i in range(2)]
            kT = [bp.tile([128, S], BF16, tag=f"kT{i}", name=f"kT{i}")
                  for i in range(2)]
            v_nat = [bp.tile([128, S], BF16, tag=f"v_nat{i}",
                             name=f"v_nat{i}") for i in range(2)]

            oT_h = [[dram.tile([HDQ, QB], BF16, name=f"oT_h{b}_{qb}")
                     for qb in range(NQB)] for b in range(B)]
            oT_F = [[dram.tile([DM, QB], BF16,
                               addr_space="Local" if sim else "Shared",
                               name=f"oT_F{b}_{qb}") for qb in range(NQB)]
                    for b in range(B)]

            env = dict(
                nc=nc, tc=tc, mybir=mybir, F32=F32, BF16=BF16,
                xg=xg, wq_sb=wq_sb, wkv_sb=wkv_sb,
                cos_sb=cos_sb, sin_sb=sin_sb, tri_sb=tri_sb, id_sb=id_sb,
                ones_sb=ones_sb, qTall=qTall, kT=kT, v_nat=v_nat,
                oT_h=oT_h, oT_F=oT_F, rg=rg, sim=sim,
                rwp=rwp, qrp=qrp, tmp=tmp, wa=wa, ptp=ptp, accp=accp,
                otsb=otsb, ywp=ywp, ps_s=ps_s, ps_o=ps_o, y=y,
            )

            with (
                tc.tile_pool(name="xgp", bufs=2) as xgp,
                tc.tile_pool(name="ps_q", bufs=1, space="PSUM") as ps_q,
                tc.tile_pool(name="ps_kv", bufs=1, space="PSUM") as ps_kv,
                tc.tile_pool(name="ps_T", bufs=1, space="PSUM") as ps_T,
            ):
                env.update(xgp=xgp, ps_q=ps_q, ps_kv=ps_kv, ps_T=ps_T)
                xg_tiles = {}
                env["xg_tiles"] = xg_tiles

                def load_xg(b, g):
                    t = xgp.tile([128, KC * 256], BF16, tag="xg")
                    r0 = (b * NG + g) * 128
                    half = KC * 128
                    eng = nc.sync if g % 2 == 0 else nc.gpsimd
                    eng.dma_start(out=t[:, 0:half],
                                  in_=xg.ap()[r0:r0 + 128, 0:half])
                    eng.dma_start(out=t[:, half:],
                                  in_=xg.ap()[r0:r0 + 128, half:])
                    xg_tiles[(b, g)] = t

                t0 = xgp.tile([128, KC * 256], BF16, tag="xg")
                q4 = KC * 64
                nc.sync.dma_start(out=t0[:, 0:q4],
                                  in_=xg.ap()[0:128, 0:q4])
                nc.gpsimd.dma_start(out=t0[:, 2 * q4:3 * q4],
                                     in_=xg.ap()[0:128, 2 * q4:3 * q4])
                nc.sync.dma_start(out=wq_sb[:, 0:2 * HDQ],
                                  in_=wqh.ap()[:, 0:2 * HDQ])
                nc.sync.dma_start(out=t0[:, q4:2 * q4],
                                  in_=xg.ap()[0:128, q4:2 * q4])
                nc.gpsimd.dma_start(out=t0[:, 3 * q4:],
                                     in_=xg.ap()[0:128, 3 * q4:])
                xg_tiles[(0, 0)] = t0
                load_xg(0, 1)
                load_weights()

                # ---- phase A: QKV b0 alone ----
                for g in range(NG):
                    _qkv_granule(env, 0, g)

                # ---- phase B: QKV b1 interleaved with attn b0 ----
                for i in range(16):
                    g, tt = i // 2, i % 2
                    _qkv_granule_half(env, 1, g, tt)
                    _attn_unit(env, 0, i // 4, i % 4)
                    if i % 4 == 3:
                        _emit_ag(env, 0, i // 4)

            # QKV psum + xg pools closed; open WO pools in freed space
            with (
                tc.tile_pool(name="stp", bufs=34) as stp,
                tc.tile_pool(name="ps_y", bufs=2, space="PSUM") as ps_y,
            ):
                env.update(wo_sb=wo_sb, stp=stp, ps_y=ps_y)
                _load_strips(env, 0, 0)

                # ---- phase C: attn b1 interleaved with WO b0 + early WO b1
                qb_order = [2, 3, 1, 0]
                wo_stream = [(0, qb, half) for qb in range(NQB)
                             for half in range(2)]
                wo_stream += [(1, qb, half) for qb in (2, 3)
                              for half in range(2)]
                wi = 0
                for i in range(16):
                    _attn_unit(env, 1, qb_order[i // 4], i % 4)
                    if i % 4 == 3:
                        _emit_ag(env, 1, qb_order[i // 4])
                    # ~12 wo half-units across 16 attn units
                    while wi < len(wo_stream) and wi * 16 <= i * 12:
                        _wo_half(env, *wo_stream[wi])
                        wi += 1
                while wi < len(wo_stream):
                    _wo_half(env, *wo_stream[wi])
                    wi += 1

                # ---- phase D: rest of WO b1 ----
                for qb in (1, 0):
                    for half in range(2):
                        _wo_half(env, 1, qb, half)

    nc.compile()
    return nc


def _qkv_granule(env, b, g):
    for tt in range(2):
        _qkv_granule_half(env, b, g, tt)


def _qkv_granule_half(env, b, g, tt):
    """One 128-token tile: q pass, kv pass, rope, transposes."""
    nc = env["nc"]
    F32, BF16 = env["F32"], env["BF16"]
    par = b % 2
    gt = g * 2 + tt                    # token tile index within batch
    pos = gt * 128

    if tt == 0:
        # prefetch granule g+3 of this batch, or early granules of b+1
        pg = g + 2
        pb = b
        if pg >= NG:
            pb, pg = b + 1, pg - NG
        if pb < B:
            xgp, xg, xg_tiles = env["xgp"], env["xg"], env["xg_tiles"]
            t = xgp.tile([128, KC * 256], BF16, tag="xg")
            r0 = (pb * NG + pg) * 128
            half = KC * 128
            eng = nc.sync if pg % 2 == 0 else nc.gpsimd
            eng.dma_start(out=t[:, 0:half], in_=xg.ap()[r0:r0 + 128, 0:half])
            eng.dma_start(out=t[:, half:], in_=xg.ap()[r0:r0 + 128, half:])
            xg_tiles[(pb, pg)] = t

    xg_t = env["xg_tiles"][(b, g)]
    wq_sb, wkv_sb = env["wq_sb"], env["wkv_sb"]

    # q pass: out [128 tok, 512 dq], accumulate over 32 chunks
    psq = env["ps_q"].tile([128, HDQ], F32, tag="psq")
    for kc in range(KC):
        nc.tensor.matmul(
            psq[:], xg_t[:, kc * 256 + tt * 128: kc * 256 + (tt + 1) * 128],
            wq_sb[:, kc * HDQ:(kc + 1) * HDQ],
            start=(kc == 0), stop=(kc == KC - 1),
        )
    # kv pass: out [128 tok, 256 (k|v)]
    pskv = env["ps_kv"].tile([128, 256], F32, tag="kv",
                             padded_shape=[128, 512])
    for kc in range(KC):
        nc.tensor.matmul(
            pskv[:], xg_t[:, kc * 256 + tt * 128: kc * 256 + (tt + 1) * 128],
            wkv_sb[:, kc * 256:(kc + 1) * 256],
            start=(kc == 0), stop=(kc == KC - 1),
        )

    # drains: copies + rope (scalar/vector), then PE transposes
    rwp, qrp, tmp = env["rwp"], env["qrp"], env["tmp"]
    cos_sb, sin_sb, id_sb = env["cos_sb"], env["sin_sb"], env["id_sb"]
    rq = rwp.tile([128, HDQ], BF16, tag="rq")
    nc.scalar.copy(rq[:], psq[:])
    rkv = rwp.tile([128, 256], BF16, tag="rkv")
    nc.scalar.copy(rkv[:], pskv[:])
    nc.scalar.copy(env["v_nat"][par][:, pos:pos + 128], rkv[:, 128:256])

    # contiguous-block rope: head slice u<64 -> x0 (even dims), u>=64 -> x1
    q_rot = qrp.tile([128, HDQ], BF16, tag="qr")
    csl = slice(gt * 256, (gt + 1) * 256)
    cosv = cos_sb[:, csl].rearrange("p (h d) -> p h d", h=NQH)
    sinv = sin_sb[:, csl].rearrange("p (h d) -> p h d", h=NQH)
    rqv = rq[:].rearrange("p (h half d) -> p h half d", h=NQH, half=2)
    rov = q_rot[:].rearrange("p (h half d) -> p h half d", h=NQH, half=2)
    x0 = rqv[:, :, 0, :]
    x1 = rqv[:, :, 1, :]
    m0 = tmp.tile([128, 256], BF16, tag="m0", name="m0")
    m1 = tmp.tile([128, 256], BF16, tag="m1", name="m1")
    m0v = m0[:].rearrange("p (h d) -> p h d", h=NQH)
    m1v = m1[:].rearrange("p (h d) -> p h d", h=NQH)
    nc.vector.tensor_mul(m0v, x0, cosv)
    nc.vector.tensor_mul(m1v, x1, sinv)
    nc.vector.tensor_sub(rov[:, :, 0, :], m0v, m1v)
    m2 = tmp.tile([128, 256], BF16, tag="m0", name="m2")
    m3 = tmp.tile([128, 256], BF16, tag="m1", name="m3")
    m2v = m2[:].rearrange("p (h d) -> p h d", h=NQH)
    m3v = m3[:].rearrange("p (h d) -> p h d", h=NQH)
    nc.vector.tensor_mul(m2v, x0, sinv)
    nc.vector.tensor_mul(m3v, x1, cosv)
    nc.vector.tensor_add(rov[:, :, 1, :], m2v, m3v)

    k_rot = qrp.tile([128, 128], BF16, tag="kr")
    kc0 = cos_sb[:, gt * 256:gt * 256 + 64]
    ks0 = sin_sb[:, gt * 256:gt * 256 + 64]
    kx0 = rkv[:, 0:64]
    kx1 = rkv[:, 64:128]
    km0 = tmp.tile([128, 64], BF16, tag="km0", name="km0")
    km1 = tmp.tile([128, 64], BF16, tag="km1", name="km1")
    nc.vector.tensor_mul(km0[:], kx0, kc0)
    nc.vector.tensor_mul(km1[:], kx1, ks0)
    nc.vector.tensor_sub(k_rot[:, 0:64], km0[:], km1[:])
    km2 = tmp.tile([128, 64], BF16, tag="km0", name="km2")
    km3 = tmp.tile([128, 64], BF16, tag="km1", name="km3")
    nc.vector.tensor_mul(km2[:], kx0, ks0)
    nc.vector.tensor_mul(km3[:], kx1, kc0)
    nc.vector.tensor_add(k_rot[:, 64:128], km2[:], km3[:])

    # transposes (q heads + k) into one packed psum bank
    tT = env["ps_T"].tile([128, 640], BF16, tag="T", padded_shape=[128, 1024])
    for h in range(NQH):
        nc.tensor.transpose(tT[:, h * 128:(h + 1) * 128],
                            q_rot[:, h * 128:(h + 1) * 128], id_sb[:])
    nc.tensor.transpose(tT[:, 512:640], k_rot[:], id_sb[:])
    nc.scalar.copy(
        env["qTall"][par][:].rearrange("p (h s) -> p h s", h=NQH)
        [:, :, pos:pos + 128],
        tT[:, 0:512].rearrange("p (h t) -> p h t", h=NQH),
    )
    nc.scalar.copy(env["kT"][par][:, pos:pos + 128], tT[:, 512:640])


def _attn_unit(env, b, qb, h):
    nc, mybir = env["nc"], env["mybir"]
    F32, BF16 = env["F32"], env["BF16"]
    par = b % 2
    qTall, kTt, v_natt = env["qTall"][par], env["kT"][par], env["v_nat"][par]
    tri_sb, ones_sb = env["tri_sb"], env["ones_sb"]
    ps_s, ps_o = env["ps_s"], env["ps_o"]
    ptp, accp, wa, otsb = env["ptp"], env["accp"], env["wa"], env["otsb"]

    q0 = qb * QB
    kt_max = (q0 + QB) // 128 - 1
    oT = ps_o.tile([128, QB], F32, tag="oT")
    acc = accp.tile([128, QB], BF16, tag="acc", name="acc")

    sTs = {}

    def emit_s(kt):
        off = max(0, kt * 128 - q0)
        qs = slice(h * S + q0 + off, h * S + q0 + QB)
        sT = ps_s.tile([128, QB], F32, tag="sT", name="sT")
        nc.tensor.matmul(
            sT[:, off:QB],
            kTt[:, kt * 128:(kt + 1) * 128],
            qTall[:, qs],
            start=True, stop=True,
        )
        sTs[kt] = (sT, off)

    emit_s(0)
    if kt_max >= 1:
        emit_s(1)
    for kt in range(kt_max + 1):
        if kt + 2 <= kt_max:
            emit_s(kt + 2)
        sT, off = sTs.pop(kt)
        psl = slice(off, QB)
        pT = ptp.tile([128, QB], BF16, tag="pT", name="pT")
        nc.scalar.activation(
            pT[:, psl], sT[:, psl],
            mybir.ActivationFunctionType.Exp,
            scale=SCALE,
        )
        if kt * 128 >= q0:
            nc.vector.tensor_mul(
                pT[:, off:off + 128],
                pT[:, off:off + 128],
                tri_sb[:],
            )
        nc.tensor.matmul(
            oT[:, psl],
            v_natt[:, kt * 128:(kt + 1) * 128],
            pT[:, psl],
            start=(kt == 0), stop=(kt == kt_max),
        )
        if kt == 0:
            nc.vector.tensor_copy(acc[:], pT[:])
        else:
            nc.vector.tensor_add(acc[:, psl], acc[:, psl], pT[:, psl])

    sums = ps_s.tile([128, QB], F32, tag="sT", name="sums")
    nc.tensor.matmul(sums[:], ones_sb[:], acc[:], start=True, stop=True)
    rec = wa.tile([128, QB], F32, tag="rec")
    scr = wa.tile([128, QB], F32, tag="scr")
    nc.vector.reciprocal_approx_accurate(rec[:], sums[:], scr[:])
    oT_sb = otsb.tile([128, QB], BF16, tag="oT_sb")
    nc.vector.tensor_mul(oT_sb[:], oT[:], rec[:])
    nc.sync.dma_start(
        out=env["oT_h"][b][qb][:][h * 128:(h + 1) * 128, :],
        in_=oT_sb[:],
    )


def _emit_ag(env, b, qb):
    nc, mybir = env["nc"], env["mybir"]
    if not env["sim"]:
        nc.gpsimd.collective_compute(
            "AllGather", mybir.AluOpType.bypass,
            replica_groups=env["rg"],
            ins=[env["oT_h"][b][qb][:].opt()],
            outs=[env["oT_F"][b][qb][:].opt()],
        )
    else:
        for c in range(N_CORES):
            nc.sync.dma_start(
                out=env["oT_F"][b][qb][:][c * HDQ:(c + 1) * HDQ, :],
                in_=env["oT_h"][b][qb][:],
            )


def _load_strips(env, b, qb, three_q=False):
    nc, BF16 = env["nc"], env["BF16"]
    stp, oT_F = env["stp"], env["oT_F"]
    key = ("strips", b, qb)
    if key not in env:
        if three_q:
            engs = [nc.sync, nc.scalar, nc.gpsimd]
        else:
            # keep the scalar queue free for exp during phase C
            engs = [nc.sync, nc.gpsimd]
        strips = []
        for hc in range(KC):
            strip = stp.tile([128, QB], BF16, tag="strip")
            engs[hc % len(engs)].dma_start(
                out=strip[:],
                in_=oT_F[b][qb][:][hc * 128:(hc + 1) * 128, :],
            )
            strips.append(strip)
        env[key] = strips
    return env[key]


def _wo_half(env, b, qb, half):
    """WO for 512 tokens of (b, qb): half 0 computes tti 0,1 and loads the
    strips; half 1 reuses the cached strips for tti 2,3."""
    nc = env["nc"]
    F32, BF16 = env["F32"], env["BF16"]
    stp, ps_y, ywp, wo_sb = env["stp"], env["ps_y"], env["ywp"], env["wo_sb"]
    y, oT_F = env["y"], env["oT_F"]

    strips = _load_strips(env, b, qb)

    psy = [ps_y.tile([128, HDQ], F32, tag="psy", name=f"psy{i}")
           for i in range(2)]
    for hc in range(KC):
        strip = strips[hc]
        for i in range(2):
            tti = half * 2 + i
            nc.tensor.matmul(
                psy[i][:],
                strip[:, tti * 128:(tti + 1) * 128],
                wo_sb[:, hc * HDQ:(hc + 1) * HDQ],
                start=(hc == 0), stop=(hc == KC - 1),
            )
    for i in range(2):
        tti = half * 2 + i
        y_sb = ywp.tile([128, HDQ], BF16, tag="y_sb")
        if i == 0:
            nc.scalar.copy(y_sb[:], psy[i][:])
        else:
            nc.vector.tensor_copy(y_sb[:], psy[i][:])
        row = b * S + qb * QB + tti * 128
        nc.sync.dma_start(out=y.ap()[row:row + 128, :], in_=y_sb[:])


def _in_maps(x, wq, wk, wv, wo):
    import concourse.mybir as mybir
    np_bf16 = mybir.dt.np(mybir.dt.bfloat16)

    x2 = np.asarray(x, dtype=np.float32).reshape(B, NG, 256, KC, 128)
    # xg[(b g) p, kc t] = x[b, g*256+t, kc*128+p]
    xgf = np.ascontiguousarray(x2.transpose(0, 1, 4, 3, 2)).reshape(
        B * NG * 128, KC * 256).astype(np_bf16)

    perm = np.concatenate([np.arange(0, HD, 2), np.arange(1, HD, 2)])
    wq = np.asarray(wq, np.float32)
    wk = np.asarray(wk, np.float32)
    wv = np.asarray(wv, np.float32)
    wo = np.asarray(wo, np.float32)
    cos4, sin4, tri, ident, ones = _consts()

    maps = []
    for c in range(N_CORES):
        wq_c = wq[:, c * HDQ:(c + 1) * HDQ].reshape(DM, NQH, HD)
        wq_c = wq_c[:, :, perm].reshape(DM, HDQ)
        # wqh[p, kc*HDQ + u] = wq_c[kc*128+p, u]
        wqh = np.ascontiguousarray(
            wq_c.reshape(KC, 128, HDQ).transpose(1, 0, 2)).reshape(
            128, KC * HDQ).astype(np_bf16)

        wk_c = wk[:, c * HD:(c + 1) * HD][:, perm]
        wv_c = wv[:, c * HD:(c + 1) * HD]
        wkv_c = np.concatenate([wk_c, wv_c], axis=1)      # [DM, 256]
        wkvh = np.ascontiguousarray(
            wkv_c.reshape(KC, 128, 256).transpose(1, 0, 2)).reshape(
            128, KC * 256).astype(np_bf16)

        wo_c = wo[:, c * HDQ:(c + 1) * HDQ]
        woh = np.ascontiguousarray(
            wo_c.reshape(KC, 128, HDQ).transpose(1, 0, 2)).reshape(
            128, KC * HDQ).astype(np_bf16)

        maps.append({
            "xg": xgf,
            "wqh": wqh, "wkvh": wkvh, "woh": woh,
            "cosc": cos4.astype(np_bf16), "sinc": sin4.astype(np_bf16),
            "tric": tri.astype(np_bf16),
            "identc": ident.astype(np_bf16), "onesc": ones.astype(np_bf16),
        })
    return maps


def kernel(x, wq, wk, wv, wo, start_pos=0, **_unused):
    from concourse import bass_utils

    assert int(np.asarray(start_pos)) == 0
    in_maps = _in_maps(x, wq, wk, wv, wo)

    if "nc" not in _CACHE:
        _CACHE["nc"] = _build()
    nc = _CACHE["nc"]

    res = bass_utils.run_bass_kernel_spmd(
        nc, in_maps, core_ids=list(range(N_CORES)),
        trace=bool(int(os.environ.get("KERNEL_TRACE", "0") or 0)),
    )
    _CACHE["last_result"] = res

    out = np.empty((T, DM), np.float32)
    for c in range(N_CORES):
        out[:, c * HDQ:(c + 1) * HDQ] = np.asarray(
            res.results[c]["y"], dtype=np.float32)
    return out.reshape(B, S, DM)
